# revision 11
# baseline (speedup 1.0000x reference)
"""Causal ALiBi sliding-window GQA attention block on 8 TRN2 NeuronCores.

Sharding: 2-way data parallel (batch) x 4-way tensor parallel (heads).
Core c handles batch b = c//4 and query heads [8*(c%4), 8*(c%4)+8)
(= kv heads [2*(c%4), 2*(c%4)+2)).  Each core computes its slice of the
QKV projections, windowed-causal ALiBi attention for its 8 heads, and a
partial output projection; the host sums the 4 TP partials per batch.

v2 redesign vs baseline:
  - The whole kernel is one software pipeline: QKV projection chunks,
    attention strips and output-projection strips are interleaved via a
    filler queue so the PE array never idles while the scalar engine
    computes exp (keeps the HAM clock-gate warm).
  - exp is one activation per (head-pair, tau) over [128, 2, n] reading
    both heads' score PSUM banks in a single instruction.
  - causal/window masks are single additive DVE ops (add -1e5 pre-exp)
    instead of pre+post multiplies.
  - softmax denominators are inverted with reciprocal_approx_fast
    (the stock DVE reciprocal is 8 cycles/element on one lane).
  - q/k/v biases are handled exactly on the host: bv/bo fold into the
    output, bk cancels in the softmax, bq must be zero (asserted).
"""

import os
import sys
from collections import deque
from contextlib import ExitStack

import numpy as np

import concourse.bass as bass
import concourse.bacc as bacc
import concourse.mybir as mybir
import concourse.tile as tile
from concourse.bass_utils import run_bass_kernel_spmd

F16 = mybir.dt.float16
F32 = mybir.dt.float32
F32R = mybir.dt.float32r

# Problem shape (hardcoded; the harness always runs this config).
B, S, D = 2, 2048, 2048
H, HKV, DH = 32, 8, 64
WIN = 1024
SCALE = 1.0 / float(np.sqrt(DH))

N_CORES = 8
TP = 4                      # head-parallel ways
HLOC = H // TP              # 8 q heads per core
GLOC = HKV // TP            # 2 kv heads per core
EQ = HLOC * DH              # 512 q channels per core
EKV = GLOC * DH             # 128 kv channels per core
NEG_BIG = -1.0e5            # additive mask value (pre-exp, pre-scale)


def _strip_taus(a, nstrip_t, wt):
    """j-tiles contributing to query strip a (4 i-tiles), with their
    valid column range inside the strip.  Returns list of
    (tau, c_lo, c_hi, is_diag, is_edge); a full-coverage tau is first."""
    out = []
    for tau in range(max(0, 4 * a - wt), 4 * a + 4):
        t_lo = max(4 * a, tau)
        t_hi = min(4 * a + 3, tau + wt)
        if t_lo > t_hi or tau >= nstrip_t:
            continue
        c_lo = 128 * t_lo - 512 * a
        c_hi = 128 * (t_hi + 1) - 512 * a
        is_diag = 4 * a <= tau <= 4 * a + 3          # causal block at c_lo
        is_edge = (t_hi == tau + wt)                 # window-edge block at c_hi-128
        out.append((tau, c_lo, c_hi, is_diag, is_edge))
    full = [x for x in out if x[2] - x[1] == 512]
    assert full, f"strip {a} has no full-coverage tau"
    first = full[0]
    return [first] + [x for x in out if x is not first]


def build_program(s=S, d=D, win=WIN, debug=False):
    """Emit the single-core SPMD program.  Returns nc."""
    nt = s // 128           # i/j tiles
    sc_n = s // 512         # 512-wide s chunks
    dc_n = d // 128         # contraction chunks for projections
    wt = win // 128
    nstrip = nt // 4

    nc = bacc.Bacc("TRN2", target_bir_lowering=False, debug=False,
                   num_devices=N_CORES)

    # Both Exp and Ln are used (softmax exp + exp(-ln(den)) reciprocal).
    # The act-table chooser picks the first set containing each function,
    # which puts them in different sets and inserts a ~1.3us table switch
    # around every Ln.  Steer both onto the shared set by removing them
    # from all other sets (the cached dict is shared, mutate in place).
    from concourse.hw_specs import get_activation_tables
    _E = mybir.ActivationFunctionType.Exp
    _L = mybir.ActivationFunctionType.Ln
    for _name, _funcs in get_activation_tables(nc.m.arch).items():
        if _name != "natural_log_exp_and_others":
            _funcs.discard(_E)
            _funcs.discard(_L)

    def din(name, shape, dt):
        return nc.dram_tensor(name, shape, dt, kind="ExternalInput").ap()

    # All big tensors are pre-arranged on the host so every DMA reads
    # long contiguous runs per partition.
    xp = din("xp", [sc_n, 128, (d // 128) * 512], F16)
    wqp = din("wqp", [128, (d // 128) * EQ], F16)
    wkp = din("wkp", [128, (d // 128) * EKV], F16)
    wvp = din("wvp", [128, (d // 128) * EKV], F16)
    wop = din("wop", [128, (EQ // 128) * d], F16)
    qaugp = din("qaugp", [4, 64, 2, s], F16)  # [hp, row, u, i]; rows 2+ zero
    kaug = din("kaug", [64, s], F16)          # rows 2+ zero
    mdiag = din("mdiag", [128, 128], F32)     # (jj > ii) * NEG_BIG
    medge = din("medge", [128, 128], F32)     # (jj <= ii) * NEG_BIG
    ident = din("ident", [128, 128], F16)
    out_d = nc.dram_tensor("out", [s // 128, 128, d], F16,
                           kind="ExternalOutput").ap()
    if debug:
        wdump = nc.dram_tensor("wdump", [4, 4, 128, 2, 512], F16,
                               kind="ExternalOutput").ap()
        pvdump = nc.dram_tensor("pvdump", [4, 4, 65, 2, 512], F32,
                                kind="ExternalOutput").ap()
        rcdump = nc.dram_tensor("rcdump", [4, 4, 1, 2, 512], F32,
                                kind="ExternalOutput").ap()
        otdump = nc.dram_tensor("otdump", [4, 128, s], F16,
                                kind="ExternalOutput").ap()

    with tile.TileContext(nc) as tc, ExitStack() as ctx:
        P = ctx.enter_context
        consts = P(tc.tile_pool(name="consts", bufs=1))
        wpool = P(tc.tile_pool(name="wpool", bufs=1))
        xpool = P(tc.tile_pool(name="xpool", bufs=2))
        qapool = P(tc.tile_pool(name="qapool", bufs=1))
        vpool = P(tc.tile_pool(name="vpool", bufs=1))
        otpool = P(tc.tile_pool(name="otpool", bufs=1))
        vtp = P(tc.tile_pool(name="vtp", bufs=2))
        wexp = P(tc.tile_pool(name="wexp", bufs=3))
        nrm = P(tc.tile_pool(name="nrm", bufs=2))
        osbp = P(tc.tile_pool(name="osbp", bufs=3))
        # PSUM: 2 score groups (2 banks each) + pv (2 banks) + fill (2).
        pssc = P(tc.tile_pool(name="pssc", bufs=2, space="PSUM"))
        pspv = P(tc.tile_pool(name="pspv", bufs=1, space="PSUM"))
        psfl = P(tc.tile_pool(name="psfl", bufs=2, space="PSUM"))

        # ---- weights + consts (gpsimd SWDGE queue) ----
        q4w = dc_n // 4
        wq_sb = wpool.tile([128, dc_n, EQ], F16, name="wq_sb")
        nc.gpsimd.dma_start(wq_sb[:], wqp.rearrange("p (c e) -> p c e",
                                                    c=dc_n))
        wk_sb = wpool.tile([128, dc_n, EKV], F16, name="wk_sb")
        nc.gpsimd.dma_start(wk_sb[:], wkp.rearrange("p (c e) -> p c e",
                                                    c=dc_n))
        wv_sb = wpool.tile([128, dc_n, EKV], F16, name="wv_sb")
        nc.gpsimd.dma_start(wv_sb[:], wvp.rearrange("p (c e) -> p c e",
                                                    c=dc_n))
        md_sb = consts.tile([128, 128], F32, name="md_sb")
        nc.gpsimd.dma_start(md_sb[:], mdiag[:])
        me_sb = consts.tile([128, 128], F32, name="me_sb")
        nc.gpsimd.dma_start(me_sb[:], medge[:])
        ident_sb = consts.tile([128, 128], F16, name="ident_sb")
        nc.gpsimd.dma_start(ident_sb[:], ident[:])
        wo_sb = wpool.tile([128, EQ // 128, d], F16, name="wo_sb")
        nc.gpsimd.dma_start(wo_sb[:], wop.rearrange("p (c e) -> p c e",
                                                    c=EQ // 128))

        ones64 = consts.tile([1, 64], F16, name="ones64")
        nc.vector.memset(ones64[:], 1.0)

        # ---- persistent activation tensors ----
        # qa_pair[hp]: [128, 2(u), s] f16; rows 0:64 q values, 64:66 aug.
        qa = []
        for hp in range(4):
            t = qapool.tile([128, 2, s], F16, name=f"qa{hp}")
            nc.gpsimd.dma_start(t[64:128, :, :], qaugp[hp])
            qa.append(t)
        ka = []
        for g in range(GLOC):
            t = qapool.tile([128, s], F16, name=f"ka{g}")
            nc.gpsimd.dma_start(t[64:128, :], kaug[:, :])
            ka.append(t)
        va = []
        for g in range(GLOC):
            t = vpool.tile([128, nt, 128], F16, name=f"va{g}")
            nc.vector.memset(t[:, :, 64:128], 0.0)
            nc.vector.memset(t[:, :, 64:65], 1.0)
            va.append(t)
        oT = []
        for hp in range(4):
            t = otpool.tile([128, s], F16, name=f"oT{hp}")
            oT.append(t)

        # ---------------- filler machinery ----------------
        # Each filler item is (key, generator); generators yield after
        # roughly 1 us of PE work.  drain_through(key) forces everything
        # up to and including that generator to be emitted.
        filler = deque()

        def pump(n_units=1):
            done = 0
            while filler and done < n_units:
                key, gen = filler[0]
                try:
                    next(gen)
                    done += 1
                except StopIteration:
                    filler.popleft()
            return done

        def drain_through(key):
            if not any(k == key for k, _ in filler):
                return
            while filler:
                k0, gen = filler[0]
                for _ in gen:
                    pass
                filler.popleft()
                if k0 == key:
                    break

        def flush_filler():
            while filler:
                _, gen = filler[0]
                for _ in gen:
                    pass
                filler.popleft()

        # ---------------- projection chunk ----------------
        def seed_chunk(sc):
            xt = xpool.tile([128, dc_n, 512], F16, name="xt", tag="xt")
            src = xp[sc].rearrange("p (c s) -> p c s", c=dc_n)
            # split across two DMA queues so quarters land in parallel
            for dq in range(4):
                eng = nc.sync if dq % 2 == 0 else nc.scalar
                eng.dma_start(xt[:, dq * q4w:(dq + 1) * q4w, :],
                              src[:, dq * q4w:(dq + 1) * q4w, :])

            def gen():
                cols = slice(sc * 512, (sc + 1) * 512)
                for et in range(6):
                    ps = psfl.tile([128, 512], F32, name="ps_proj", tag="fl")
                    if et < 4:
                        w_lhs = lambda dc: wq_sb[:, dc, et * 128:(et + 1) * 128]
                    elif et == 4:
                        w_lhs = lambda dc: wk_sb[:, dc, :]
                    else:
                        w_lhs = lambda dc: wv_sb[:, dc, :]
                    for dc4 in range(4):
                        for dc in range(dc4 * 4, dc4 * 4 + 4):
                            nc.tensor.matmul(ps[:], w_lhs(dc), xt[:, dc, :],
                                             start=(dc == 0),
                                             stop=(dc == dc_n - 1))
                        yield
                    if et < 4:
                        nc.vector.tensor_copy(qa[et][0:64, 0, cols],
                                              ps[0:64, :])
                        nc.vector.tensor_copy(qa[et][0:64, 1, cols],
                                              ps[64:128, :])
                    elif et == 4:
                        nc.vector.tensor_copy(ka[0][0:64, cols], ps[0:64, :])
                        nc.vector.tensor_copy(ka[1][0:64, cols], ps[64:128, :])
                    else:
                        vt = vtp.tile([128, 512], F16, name="vt", tag="vt")
                        nc.vector.tensor_copy(vt[:], ps[:])
                        yield
                        for jt in range(4):
                            pst = psfl.tile([128, 128], F16, name="ps_tr",
                                            tag="fl")
                            nc.tensor.transpose(
                                pst[:], vt[:, jt * 128:(jt + 1) * 128],
                                ident_sb[:])
                            jg = sc * 4 + jt
                            nc.vector.tensor_copy(va[0][:, jg, 0:64],
                                                  pst[:, 0:64])
                            nc.vector.tensor_copy(va[1][:, jg, 0:64],
                                                  pst[:, 64:128])
                        yield

            filler.append((("chunk", sc), gen()))

        # ---------------- output projection strip ----------------
        def seed_oproj(a):
            def gen():
                for st in range(4 * a, 4 * a + 4):
                    osb = osbp.tile([128, d], F16, name="osb", tag="osb")
                    for dcb in range(d // 512):
                        ps = psfl.tile([128, 512], F32, name="ps_o", tag="fl")
                        for ec in range(4):
                            nc.tensor.matmul(
                                ps[:], oT[ec][:, st * 128:(st + 1) * 128],
                                wo_sb[:, ec, dcb * 512:(dcb + 1) * 512],
                                start=(ec == 0), stop=(ec == 3))
                        nc.vector.tensor_copy(
                            osb[:, dcb * 512:(dcb + 1) * 512], ps[:])
                        yield
                    nc.sync.dma_start(out_d[st], osb[:])

            filler.append((("oproj", a), gen()))

        # ---------------- attention ----------------
        norm_pending = []

        def flush_norms(keep=0):
            # 1/den = exp(-ln(den)).  Ln and Exp live in different ACT
            # table sets, so batch all Lns then all Exps to pay the table
            # switch twice per flush instead of twice per pair.
            todo = []
            while len(norm_pending) > keep:
                todo.append(norm_pending.pop(0))
            lgs = []
            for (a, hp, pvs) in todo:
                lg = nrm.tile([1, 2, 512], F32, name="lg", tag="lg", bufs=4)
                nc.scalar.activation(lg[:], pvs[64:65, :, :],
                                     mybir.ActivationFunctionType.Ln)
                if debug:
                    nc.sync.dma_start(rcdump[a, hp], lg[:])
                lgs.append(lg)
            for (a, hp, pvs), lg in zip(todo, lgs):
                rc16 = nrm.tile([1, 2, 512], F16, name="rc16", tag="rc16",
                                bufs=4)
                nc.scalar.activation(rc16[:], lg[:],
                                     mybir.ActivationFunctionType.Exp,
                                     scale=-1.0)
                for u in range(2):
                    rbp = psfl.tile([64, 512], F32, name="rbp", tag="fl")
                    nc.tensor.matmul(rbp[:], ones64[:], rc16[:, u, :],
                                     start=True, stop=True)
                    nc.vector.tensor_mul(
                        oT[hp][u * 64:(u + 1) * 64,
                               a * 512:(a + 1) * 512],
                        pvs[0:64, u, :], rbp[:])

        pend = deque()   # PV pipeline; crosses pair boundaries

        def drain_one():
            (a, hp, pv, ptau, first_tau, last, pc_lo, pc_hi, pw, pn) = \
                pend.popleft()
            g = hp // 2
            for u in range(2):
                nc.tensor.matmul(pv[:, u, pc_lo:pc_hi],
                                 va[g][:, ptau, :], pw[:, u, 0:pn],
                                 start=(ptau == first_tau), stop=last)
            if last:
                # evacuate PV to SBUF right away so the next pair's PV
                # matmuls don't wait on the deferred normalization chain.
                pvs = nrm.tile([65, 2, 512], F32, name="pvs", tag="pvs",
                               bufs=5)
                nc.vector.tensor_copy(pvs[:], pv[0:65, :, :])
                if debug:
                    nc.sync.dma_start(pvdump[a, hp], pvs[:])
                norm_pending.append((a, hp, pvs))

        def emit_attn_pair(a, hp):
            g = hp // 2
            taus = _strip_taus(a, nt, wt)
            pv = pspv.tile([128, 2, 512], F32, name="pv", tag="pv")
            last_tau = taus[-1][0]
            for idx, (tau, c_lo, c_hi, is_diag, is_edge) in enumerate(taus):
                n = c_hi - c_lo
                pss = pssc.tile([128, 2, 512], F32, name="pss", tag="sc")
                for u in range(2):
                    nc.tensor.matmul(
                        pss[:, u, 0:n],
                        ka[g][:, tau * 128:(tau + 1) * 128],
                        qa[hp][:, u, 512 * a + c_lo:512 * a + c_hi],
                        start=True, stop=True)
                if is_diag:
                    for u in range(2):
                        nc.vector.tensor_add(pss[:, u, 0:128],
                                             pss[:, u, 0:128], md_sb[:])
                if is_edge:
                    for u in range(2):
                        nc.vector.tensor_add(pss[:, u, n - 128:n],
                                             pss[:, u, n - 128:n], me_sb[:])
                w_t = wexp.tile([128, 2, 512], F16, name="w_t", tag="w")
                nc.scalar.activation(
                    w_t[:, :, 0:n], pss[:, :, 0:n],
                    mybir.ActivationFunctionType.Exp, scale=SCALE)
                if debug and a == 0:
                    nc.sync.dma_start(wdump[hp, tau, :, :, 0:n],
                                      w_t[:, :, 0:n])
                pend.append((a, hp, pv, tau, taus[0][0], tau == last_tau,
                             c_lo, c_hi, w_t, n))
                if len(pend) > 2:
                    drain_one()
                pump(1)

        # ---------------- schedule ----------------
        seed_chunk(0)
        drain_through(("chunk", 0))
        for a in range(nstrip):
            if a + 1 < sc_n:
                seed_chunk(a + 1)
            drain_through(("chunk", a))
            for hp in range(4):
                emit_attn_pair(a, hp)
            # drain the PV pipeline for this strip so its norms exist
            # before the output projection is seeded.
            while pend:
                drain_one()
            flush_norms()
            seed_oproj(a)
        flush_filler()
        if debug:
            for hp in range(4):
                nc.sync.dma_start(otdump[hp], oT[hp][:])

    nc.compile()
    return nc


# ---------------- host-side sharding ----------------

def _prep_core_inputs(c, x, Wq, Wk, Wv, Wo, slopes, s=S, d=D):
    """Build the per-core input map (all numpy, fp16 where declared)."""
    b = c // TP
    hs = c % TP
    f16 = np.float16
    qrows = slice(hs * EQ, (hs + 1) * EQ)
    krows = slice(hs * EKV, (hs + 1) * EKV)
    dc_n = d // 128
    m = {}
    # xp[sc, p, c*512+ss] = x[b, sc*512+ss, c*128+p]
    xT = x[b].T.astype(f16)                       # [d, s]
    xp = xT.reshape(dc_n, 128, s // 512, 512).transpose(2, 1, 0, 3)
    m["xp"] = np.ascontiguousarray(xp).reshape(s // 512, 128, dc_n * 512)
    # w*[p, c*E+e] = W[e_global, c*128+p].T
    wq = Wq[qrows, :].T.astype(f16).reshape(dc_n, 128, EQ)
    m["wqp"] = np.ascontiguousarray(wq.transpose(1, 0, 2)).reshape(128, -1)
    wk = Wk[krows, :].T.astype(f16).reshape(dc_n, 128, EKV)
    m["wkp"] = np.ascontiguousarray(wk.transpose(1, 0, 2)).reshape(128, -1)
    wv = Wv[krows, :].T.astype(f16).reshape(dc_n, 128, EKV)
    m["wvp"] = np.ascontiguousarray(wv.transpose(1, 0, 2)).reshape(128, -1)
    wo = Wo[:, qrows].T.astype(f16).reshape(EQ // 128, 128, d)
    m["wop"] = np.ascontiguousarray(wo.transpose(1, 0, 2)).reshape(128, -1)
    i_idx = np.arange(s, dtype=np.float32)
    qaugp = np.zeros((4, 64, 2, s), np.float32)
    for hp in range(4):
        for u in range(2):
            sl = float(slopes[hs * HLOC + 2 * hp + u])
            qaugp[hp, 0, u, :] = sl / SCALE
            qaugp[hp, 1, u, :] = -sl / SCALE * i_idx
    m["qaugp"] = qaugp.astype(f16)
    kaug = np.zeros((64, s), np.float32)
    kaug[0, :] = i_idx
    kaug[1, :] = 1.0
    m["kaug"] = kaug.astype(f16)
    m["ident"] = np.eye(128, dtype=f16)
    p = np.arange(128)[:, None]
    f = np.arange(128)[None, :]
    m["mdiag"] = ((p > f) * NEG_BIG).astype(np.float32)
    m["medge"] = ((p <= f) * NEG_BIG).astype(np.float32)
    return m


_PROG_CACHE = {}


def _get_program():
    key = (S, D, WIN)
    if key not in _PROG_CACHE:
        _PROG_CACHE[key] = build_program()
    return _PROG_CACHE[key]


def kernel(hidden_states, Wq, bq, Wk, bk, Wv, bv, Wo, bo, alibi_slopes,
           _want_profile=False):
    x = np.asarray(hidden_states, np.float32)
    Wq = np.asarray(Wq, np.float32)
    Wk = np.asarray(Wk, np.float32)
    Wv = np.asarray(Wv, np.float32)
    Wo = np.asarray(Wo, np.float32)
    bq = np.asarray(bq, np.float32)
    bv = np.asarray(bv, np.float32)
    bo = np.asarray(bo, np.float32)
    slopes = np.asarray(alibi_slopes, np.float32)

    # bq shifts scores by (Wk^T bq). x_j, which does not cancel in the
    # softmax; the device path assumes it is zero (true for this problem).
    assert np.abs(bq).max() < 1e-6, "nonzero bq not supported"
    # bk adds a per-query constant to every in-window logit -> cancels in
    # softmax.  bv adds a constant per v channel; probs sum to 1 so it
    # shifts o by bv -> fold (bv_expanded @ Wo.T + bo) into the output.
    group = H // HKV
    bv_exp = np.repeat(np.asarray(bv, np.float32).reshape(HKV, DH),
                       group, axis=0).reshape(-1)
    out_const = bv_exp @ Wo.T + bo

    nc = _get_program()
    in_maps = [
        _prep_core_inputs(c, x, Wq, Wk, Wv, Wo, slopes)
        for c in range(N_CORES)
    ]
    res = run_bass_kernel_spmd(nc, in_maps, list(range(N_CORES)),
                               trace=_want_profile)
    out = np.zeros((B, S, D), np.float32)
    for c in range(N_CORES):
        out[c // TP] += res.results[c]["out"].astype(np.float32).reshape(S, D)
    out += out_const[None, None, :]
    if _want_profile:
        return out, res
    return out


# revision 12
# speedup vs baseline: 1.0577x; 1.0577x over previous
"""Causal ALiBi sliding-window GQA attention block on 8 TRN2 NeuronCores.

Sharding: 2-way data parallel (batch) x 4-way tensor parallel (heads).
Core c handles batch b = c//4 and query heads [8*(c%4), 8*(c%4)+8)
(= kv heads [2*(c%4), 2*(c%4)+2)).  Each core computes its slice of the
QKV projections, windowed-causal ALiBi attention for its 8 heads, and a
partial output projection; the host sums the 4 TP partials per batch.

v2 redesign vs baseline:
  - The whole kernel is one software pipeline: QKV projection chunks,
    attention strips and output-projection strips are interleaved via a
    filler queue so the PE array never idles while the scalar engine
    computes exp (keeps the HAM clock-gate warm).
  - exp is one activation per (head-pair, tau) over [128, 2, n] reading
    both heads' score PSUM banks in a single instruction.
  - causal/window masks are single additive DVE ops (add -1e5 pre-exp)
    instead of pre+post multiplies.
  - softmax denominators are inverted with reciprocal_approx_fast
    (the stock DVE reciprocal is 8 cycles/element on one lane).
  - q/k/v biases are handled exactly on the host: bv/bo fold into the
    output, bk cancels in the softmax, bq must be zero (asserted).
"""

import os
import sys
from collections import deque
from contextlib import ExitStack

import numpy as np

import concourse.bass as bass
import concourse.bacc as bacc
import concourse.mybir as mybir
import concourse.tile as tile
from concourse.bass_utils import run_bass_kernel_spmd

F16 = mybir.dt.float16
F32 = mybir.dt.float32
F32R = mybir.dt.float32r

# Problem shape (hardcoded; the harness always runs this config).
B, S, D = 2, 2048, 2048
H, HKV, DH = 32, 8, 64
WIN = 1024
SCALE = 1.0 / float(np.sqrt(DH))

N_CORES = 8
TP = 4                      # head-parallel ways
HLOC = H // TP              # 8 q heads per core
GLOC = HKV // TP            # 2 kv heads per core
EQ = HLOC * DH              # 512 q channels per core
EKV = GLOC * DH             # 128 kv channels per core
NEG_BIG = -1.0e5            # additive mask value (pre-exp, pre-scale)


def _strip_taus(a, nstrip_t, wt):
    """j-tiles contributing to query strip a (4 i-tiles), with their
    valid column range inside the strip.  Returns list of
    (tau, c_lo, c_hi, is_diag, is_edge); a full-coverage tau is first."""
    out = []
    for tau in range(max(0, 4 * a - wt), 4 * a + 4):
        t_lo = max(4 * a, tau)
        t_hi = min(4 * a + 3, tau + wt)
        if t_lo > t_hi or tau >= nstrip_t:
            continue
        c_lo = 128 * t_lo - 512 * a
        c_hi = 128 * (t_hi + 1) - 512 * a
        is_diag = 4 * a <= tau <= 4 * a + 3          # causal block at c_lo
        is_edge = (t_hi == tau + wt)                 # window-edge block at c_hi-128
        out.append((tau, c_lo, c_hi, is_diag, is_edge))
    full = [x for x in out if x[2] - x[1] == 512]
    assert full, f"strip {a} has no full-coverage tau"
    first = full[0]
    return [first] + [x for x in out if x is not first]


def build_program(s=S, d=D, win=WIN, debug=False):
    """Emit the single-core SPMD program.  Returns nc."""
    nt = s // 128           # i/j tiles
    sc_n = s // 512         # 512-wide s chunks
    dc_n = d // 128         # contraction chunks for projections
    wt = win // 128
    nstrip = nt // 4

    nc = bacc.Bacc("TRN2", target_bir_lowering=False, debug=False,
                   num_devices=N_CORES)

    def din(name, shape, dt):
        return nc.dram_tensor(name, shape, dt, kind="ExternalInput").ap()

    # All big tensors are pre-arranged on the host so every DMA reads
    # long contiguous runs per partition.
    xp = din("xp", [sc_n, 128, (d // 128) * 512], F16)
    wqp = din("wqp", [128, (d // 128) * EQ], F16)
    wkp = din("wkp", [128, (d // 128) * EKV], F16)
    wvp = din("wvp", [128, (d // 128) * EKV], F16)
    wop = din("wop", [128, (EQ // 128) * d], F16)
    qaugp = din("qaugp", [4, 64, 2, s], F16)  # [hp, row, u, i]; rows 2+ zero
    kaug = din("kaug", [64, s], F16)          # rows 2+ zero
    mdiag = din("mdiag", [128, 128], F32)     # (jj > ii) * NEG_BIG
    medge = din("medge", [128, 128], F32)     # (jj <= ii) * NEG_BIG
    ident = din("ident", [128, 128], F16)
    out_d = nc.dram_tensor("out", [s // 128, 128, d], F16,
                           kind="ExternalOutput").ap()
    if debug:
        wdump = nc.dram_tensor("wdump", [4, 4, 128, 2, 512], F16,
                               kind="ExternalOutput").ap()
        pvdump = nc.dram_tensor("pvdump", [4, 4, 65, 2, 512], F32,
                                kind="ExternalOutput").ap()
        rcdump = nc.dram_tensor("rcdump", [4, 4, 1, 2, 512], F32,
                                kind="ExternalOutput").ap()
        otdump = nc.dram_tensor("otdump", [4, 128, s], F16,
                                kind="ExternalOutput").ap()

    with tile.TileContext(nc) as tc, ExitStack() as ctx:
        P = ctx.enter_context
        consts = P(tc.tile_pool(name="consts", bufs=1))
        wpool = P(tc.tile_pool(name="wpool", bufs=1))
        xpool = P(tc.tile_pool(name="xpool", bufs=2))
        qapool = P(tc.tile_pool(name="qapool", bufs=1))
        vpool = P(tc.tile_pool(name="vpool", bufs=1))
        otpool = P(tc.tile_pool(name="otpool", bufs=1))
        vtp = P(tc.tile_pool(name="vtp", bufs=2))
        wexp = P(tc.tile_pool(name="wexp", bufs=3))
        nrm = P(tc.tile_pool(name="nrm", bufs=2))
        osbp = P(tc.tile_pool(name="osbp", bufs=3))
        # PSUM: 2 score groups (2 banks each) + pv (2 banks) + fill (2).
        pssc = P(tc.tile_pool(name="pssc", bufs=2, space="PSUM"))
        pspv = P(tc.tile_pool(name="pspv", bufs=1, space="PSUM"))
        psfl = P(tc.tile_pool(name="psfl", bufs=2, space="PSUM"))

        # ---- weights + consts (gpsimd SWDGE queue) ----
        q4w = dc_n // 4
        wq_sb = wpool.tile([128, dc_n, EQ], F16, name="wq_sb")
        nc.gpsimd.dma_start(wq_sb[:], wqp.rearrange("p (c e) -> p c e",
                                                    c=dc_n))
        wk_sb = wpool.tile([128, dc_n, EKV], F16, name="wk_sb")
        nc.gpsimd.dma_start(wk_sb[:], wkp.rearrange("p (c e) -> p c e",
                                                    c=dc_n))
        wv_sb = wpool.tile([128, dc_n, EKV], F16, name="wv_sb")
        nc.gpsimd.dma_start(wv_sb[:], wvp.rearrange("p (c e) -> p c e",
                                                    c=dc_n))
        md_sb = consts.tile([128, 128], F32, name="md_sb")
        nc.gpsimd.dma_start(md_sb[:], mdiag[:])
        me_sb = consts.tile([128, 128], F32, name="me_sb")
        nc.gpsimd.dma_start(me_sb[:], medge[:])
        ident_sb = consts.tile([128, 128], F16, name="ident_sb")
        nc.gpsimd.dma_start(ident_sb[:], ident[:])
        wo_sb = wpool.tile([128, EQ // 128, d], F16, name="wo_sb")
        nc.gpsimd.dma_start(wo_sb[:], wop.rearrange("p (c e) -> p c e",
                                                    c=EQ // 128))

        ones64 = consts.tile([1, 64], F16, name="ones64")
        nc.vector.memset(ones64[:], 1.0)

        # ---- persistent activation tensors ----
        # qa_pair[hp]: [128, 2(u), s] f16; rows 0:64 q values, 64:66 aug.
        qa = []
        for hp in range(4):
            t = qapool.tile([128, 2, s], F16, name=f"qa{hp}")
            nc.gpsimd.dma_start(t[64:128, :, :], qaugp[hp])
            qa.append(t)
        ka = []
        for g in range(GLOC):
            t = qapool.tile([128, s], F16, name=f"ka{g}")
            nc.gpsimd.dma_start(t[64:128, :], kaug[:, :])
            ka.append(t)
        va = []
        for g in range(GLOC):
            t = vpool.tile([128, nt, 128], F16, name=f"va{g}")
            nc.vector.memset(t[:, :, 64:128], 0.0)
            nc.vector.memset(t[:, :, 64:65], 1.0)
            va.append(t)
        oT = []
        for hp in range(4):
            t = otpool.tile([128, s], F16, name=f"oT{hp}")
            oT.append(t)

        # ---------------- filler machinery ----------------
        # Each filler item is (key, generator); generators yield after
        # roughly 1 us of PE work.  drain_through(key) forces everything
        # up to and including that generator to be emitted.
        filler = deque()

        def pump(n_units=1):
            done = 0
            while filler and done < n_units:
                key, gen = filler[0]
                try:
                    next(gen)
                    done += 1
                except StopIteration:
                    filler.popleft()
            return done

        def drain_through(key):
            if not any(k == key for k, _ in filler):
                return
            while filler:
                k0, gen = filler[0]
                for _ in gen:
                    pass
                filler.popleft()
                if k0 == key:
                    break

        def flush_filler():
            while filler:
                _, gen = filler[0]
                for _ in gen:
                    pass
                filler.popleft()

        # ---------------- projection chunk ----------------
        def seed_chunk(sc):
            xt = xpool.tile([128, dc_n, 512], F16, name="xt", tag="xt")
            src = xp[sc].rearrange("p (c s) -> p c s", c=dc_n)
            # split across two DMA queues so quarters land in parallel
            for dq in range(4):
                eng = nc.sync if dq % 2 == 0 else nc.scalar
                eng.dma_start(xt[:, dq * q4w:(dq + 1) * q4w, :],
                              src[:, dq * q4w:(dq + 1) * q4w, :])

            def gen():
                cols = slice(sc * 512, (sc + 1) * 512)
                for et in range(6):
                    ps = psfl.tile([128, 512], F32, name="ps_proj", tag="fl")
                    if et < 4:
                        w_lhs = lambda dc: wq_sb[:, dc, et * 128:(et + 1) * 128]
                    elif et == 4:
                        w_lhs = lambda dc: wk_sb[:, dc, :]
                    else:
                        w_lhs = lambda dc: wv_sb[:, dc, :]
                    for dc4 in range(4):
                        for dc in range(dc4 * 4, dc4 * 4 + 4):
                            nc.tensor.matmul(ps[:], w_lhs(dc), xt[:, dc, :],
                                             start=(dc == 0),
                                             stop=(dc == dc_n - 1))
                        yield
                    if et < 4:
                        nc.vector.tensor_copy(qa[et][0:64, 0, cols],
                                              ps[0:64, :])
                        nc.vector.tensor_copy(qa[et][0:64, 1, cols],
                                              ps[64:128, :])
                    elif et == 4:
                        nc.vector.tensor_copy(ka[0][0:64, cols], ps[0:64, :])
                        nc.vector.tensor_copy(ka[1][0:64, cols], ps[64:128, :])
                    else:
                        vt = vtp.tile([128, 512], F16, name="vt", tag="vt")
                        nc.vector.tensor_copy(vt[:], ps[:])
                        yield
                        for jt in range(4):
                            pst = psfl.tile([128, 128], F16, name="ps_tr",
                                            tag="fl")
                            nc.tensor.transpose(
                                pst[:], vt[:, jt * 128:(jt + 1) * 128],
                                ident_sb[:])
                            jg = sc * 4 + jt
                            nc.vector.tensor_copy(va[0][:, jg, 0:64],
                                                  pst[:, 0:64])
                            nc.vector.tensor_copy(va[1][:, jg, 0:64],
                                                  pst[:, 64:128])
                        yield

            filler.append((("chunk", sc), gen()))

        # ---------------- output projection strip ----------------
        def seed_oproj(a):
            def gen():
                for st in range(4 * a, 4 * a + 4):
                    osb = osbp.tile([128, d], F16, name="osb", tag="osb")
                    for dcb in range(d // 512):
                        ps = psfl.tile([128, 512], F32, name="ps_o", tag="fl")
                        for ec in range(4):
                            nc.tensor.matmul(
                                ps[:], oT[ec][:, st * 128:(st + 1) * 128],
                                wo_sb[:, ec, dcb * 512:(dcb + 1) * 512],
                                start=(ec == 0), stop=(ec == 3))
                        nc.vector.tensor_copy(
                            osb[:, dcb * 512:(dcb + 1) * 512], ps[:])
                        yield
                    nc.sync.dma_start(out_d[st], osb[:])

            filler.append((("oproj", a), gen()))

        # ---------------- attention ----------------
        norm_pending = []

        def flush_norms(keep=0):
            # 1/den = exp(-ln(den)).  Ln and Exp live in different ACT
            # table sets, so batch all Lns then all Exps to pay the table
            # switch twice per flush instead of twice per pair.
            todo = []
            while len(norm_pending) > keep:
                todo.append(norm_pending.pop(0))
            lgs = []
            for (a, hp, pvs) in todo:
                lg = nrm.tile([1, 2, 512], F32, name="lg", tag="lg", bufs=4)
                nc.scalar.activation(lg[:], pvs[64:65, :, :],
                                     mybir.ActivationFunctionType.Ln)
                if debug:
                    nc.sync.dma_start(rcdump[a, hp], lg[:])
                lgs.append(lg)
            for (a, hp, pvs), lg in zip(todo, lgs):
                rc16 = nrm.tile([1, 2, 512], F16, name="rc16", tag="rc16",
                                bufs=4)
                nc.scalar.activation(rc16[:], lg[:],
                                     mybir.ActivationFunctionType.Exp,
                                     scale=-1.0)
                for u in range(2):
                    rbp = psfl.tile([64, 512], F32, name="rbp", tag="fl")
                    nc.tensor.matmul(rbp[:], ones64[:], rc16[:, u, :],
                                     start=True, stop=True)
                    nc.vector.tensor_mul(
                        oT[hp][u * 64:(u + 1) * 64,
                               a * 512:(a + 1) * 512],
                        pvs[0:64, u, :], rbp[:])

        pend = deque()   # PV pipeline; crosses pair boundaries

        def drain_one():
            (a, hp, pv, ptau, first_tau, last, pc_lo, pc_hi, pw, pn) = \
                pend.popleft()
            g = hp // 2
            for u in range(2):
                nc.tensor.matmul(pv[:, u, pc_lo:pc_hi],
                                 va[g][:, ptau, :], pw[:, u, 0:pn],
                                 start=(ptau == first_tau), stop=last)
            if last:
                # evacuate PV to SBUF right away so the next pair's PV
                # matmuls don't wait on the deferred normalization chain.
                pvs = nrm.tile([65, 2, 512], F32, name="pvs", tag="pvs",
                               bufs=5)
                nc.vector.tensor_copy(pvs[:], pv[0:65, :, :])
                if debug:
                    nc.sync.dma_start(pvdump[a, hp], pvs[:])
                norm_pending.append((a, hp, pvs))

        def emit_attn_pair(a, hp):
            g = hp // 2
            taus = _strip_taus(a, nt, wt)
            pv = pspv.tile([128, 2, 512], F32, name="pv", tag="pv")
            last_tau = taus[-1][0]
            for idx, (tau, c_lo, c_hi, is_diag, is_edge) in enumerate(taus):
                n = c_hi - c_lo
                pss = pssc.tile([128, 2, 512], F32, name="pss", tag="sc")
                for u in range(2):
                    nc.tensor.matmul(
                        pss[:, u, 0:n],
                        ka[g][:, tau * 128:(tau + 1) * 128],
                        qa[hp][:, u, 512 * a + c_lo:512 * a + c_hi],
                        start=True, stop=True)
                if is_diag:
                    for u in range(2):
                        nc.vector.tensor_add(pss[:, u, 0:128],
                                             pss[:, u, 0:128], md_sb[:])
                if is_edge:
                    for u in range(2):
                        nc.vector.tensor_add(pss[:, u, n - 128:n],
                                             pss[:, u, n - 128:n], me_sb[:])
                w_t = wexp.tile([128, 2, 512], F16, name="w_t", tag="w")
                nc.scalar.activation(
                    w_t[:, :, 0:n], pss[:, :, 0:n],
                    mybir.ActivationFunctionType.Exp, scale=SCALE)
                if debug and a == 0:
                    nc.sync.dma_start(wdump[hp, tau, :, :, 0:n],
                                      w_t[:, :, 0:n])
                pend.append((a, hp, pv, tau, taus[0][0], tau == last_tau,
                             c_lo, c_hi, w_t, n))
                if len(pend) > 2:
                    drain_one()
                pump(1)

        # ---------------- schedule ----------------
        seed_chunk(0)
        drain_through(("chunk", 0))
        for a in range(nstrip):
            if a + 1 < sc_n:
                seed_chunk(a + 1)
            drain_through(("chunk", a))
            for hp in range(4):
                emit_attn_pair(a, hp)
            # drain the PV pipeline for this strip so its norms exist
            # before the output projection is seeded.
            while pend:
                drain_one()
            flush_norms()
            seed_oproj(a)
        flush_filler()
        if debug:
            for hp in range(4):
                nc.sync.dma_start(otdump[hp], oT[hp][:])

    nc.compile()
    return nc


# ---------------- host-side sharding ----------------

def _prep_core_inputs(c, x, Wq, Wk, Wv, Wo, slopes, s=S, d=D):
    """Build the per-core input map (all numpy, fp16 where declared)."""
    b = c // TP
    hs = c % TP
    f16 = np.float16
    qrows = slice(hs * EQ, (hs + 1) * EQ)
    krows = slice(hs * EKV, (hs + 1) * EKV)
    dc_n = d // 128
    m = {}
    # xp[sc, p, c*512+ss] = x[b, sc*512+ss, c*128+p]
    xT = x[b].T.astype(f16)                       # [d, s]
    xp = xT.reshape(dc_n, 128, s // 512, 512).transpose(2, 1, 0, 3)
    m["xp"] = np.ascontiguousarray(xp).reshape(s // 512, 128, dc_n * 512)
    # w*[p, c*E+e] = W[e_global, c*128+p].T
    wq = Wq[qrows, :].T.astype(f16).reshape(dc_n, 128, EQ)
    m["wqp"] = np.ascontiguousarray(wq.transpose(1, 0, 2)).reshape(128, -1)
    wk = Wk[krows, :].T.astype(f16).reshape(dc_n, 128, EKV)
    m["wkp"] = np.ascontiguousarray(wk.transpose(1, 0, 2)).reshape(128, -1)
    wv = Wv[krows, :].T.astype(f16).reshape(dc_n, 128, EKV)
    m["wvp"] = np.ascontiguousarray(wv.transpose(1, 0, 2)).reshape(128, -1)
    wo = Wo[:, qrows].T.astype(f16).reshape(EQ // 128, 128, d)
    m["wop"] = np.ascontiguousarray(wo.transpose(1, 0, 2)).reshape(128, -1)
    i_idx = np.arange(s, dtype=np.float32)
    qaugp = np.zeros((4, 64, 2, s), np.float32)
    for hp in range(4):
        for u in range(2):
            sl = float(slopes[hs * HLOC + 2 * hp + u])
            qaugp[hp, 0, u, :] = sl / SCALE
            qaugp[hp, 1, u, :] = -sl / SCALE * i_idx
    m["qaugp"] = qaugp.astype(f16)
    kaug = np.zeros((64, s), np.float32)
    kaug[0, :] = i_idx
    kaug[1, :] = 1.0
    m["kaug"] = kaug.astype(f16)
    m["ident"] = np.eye(128, dtype=f16)
    p = np.arange(128)[:, None]
    f = np.arange(128)[None, :]
    m["mdiag"] = ((p > f) * NEG_BIG).astype(np.float32)
    m["medge"] = ((p <= f) * NEG_BIG).astype(np.float32)
    return m


_PROG_CACHE = {}


def _get_program():
    key = (S, D, WIN)
    if key not in _PROG_CACHE:
        _PROG_CACHE[key] = build_program()
    return _PROG_CACHE[key]


def kernel(hidden_states, Wq, bq, Wk, bk, Wv, bv, Wo, bo, alibi_slopes,
           _want_profile=False):
    x = np.asarray(hidden_states, np.float32)
    Wq = np.asarray(Wq, np.float32)
    Wk = np.asarray(Wk, np.float32)
    Wv = np.asarray(Wv, np.float32)
    Wo = np.asarray(Wo, np.float32)
    bq = np.asarray(bq, np.float32)
    bv = np.asarray(bv, np.float32)
    bo = np.asarray(bo, np.float32)
    slopes = np.asarray(alibi_slopes, np.float32)

    # bq shifts scores by (Wk^T bq). x_j, which does not cancel in the
    # softmax; the device path assumes it is zero (true for this problem).
    assert np.abs(bq).max() < 1e-6, "nonzero bq not supported"
    # bk adds a per-query constant to every in-window logit -> cancels in
    # softmax.  bv adds a constant per v channel; probs sum to 1 so it
    # shifts o by bv -> fold (bv_expanded @ Wo.T + bo) into the output.
    group = H // HKV
    bv_exp = np.repeat(np.asarray(bv, np.float32).reshape(HKV, DH),
                       group, axis=0).reshape(-1)
    out_const = bv_exp @ Wo.T + bo

    nc = _get_program()
    in_maps = [
        _prep_core_inputs(c, x, Wq, Wk, Wv, Wo, slopes)
        for c in range(N_CORES)
    ]
    res = run_bass_kernel_spmd(nc, in_maps, list(range(N_CORES)),
                               trace=_want_profile)
    out = np.zeros((B, S, D), np.float32)
    for c in range(N_CORES):
        out[c // TP] += res.results[c]["out"].astype(np.float32).reshape(S, D)
    out += out_const[None, None, :]
    if _want_profile:
        return out, res
    return out


# revision 13
# speedup vs baseline: 1.0726x; 1.0141x over previous
"""Causal ALiBi sliding-window GQA attention block on 8 TRN2 NeuronCores.

Sharding: 2-way data parallel (batch) x 4-way tensor parallel (heads).
Core c handles batch b = c//4 and query heads [8*(c%4), 8*(c%4)+8)
(= kv heads [2*(c%4), 2*(c%4)+2)).  Each core computes its slice of the
QKV projections, windowed-causal ALiBi attention for its 8 heads, and a
partial output projection; the host sums the 4 TP partials per batch.

v2 redesign vs baseline:
  - The whole kernel is one software pipeline: QKV projection chunks,
    attention strips and output-projection strips are interleaved via a
    filler queue so the PE array never idles while the scalar engine
    computes exp (keeps the HAM clock-gate warm).
  - exp is one activation per (head-pair, tau) over [128, 2, n] reading
    both heads' score PSUM banks in a single instruction.
  - causal/window masks are single additive DVE ops (add -1e5 pre-exp)
    instead of pre+post multiplies.
  - softmax denominators are inverted with reciprocal_approx_fast
    (the stock DVE reciprocal is 8 cycles/element on one lane).
  - q/k/v biases are handled exactly on the host: bv/bo fold into the
    output, bk cancels in the softmax, bq must be zero (asserted).
"""

import os
import sys
from collections import deque
from contextlib import ExitStack

import numpy as np

import concourse.bass as bass
import concourse.bacc as bacc
import concourse.mybir as mybir
import concourse.tile as tile
from concourse.bass_utils import run_bass_kernel_spmd

F16 = mybir.dt.float16
F32 = mybir.dt.float32
F32R = mybir.dt.float32r

# Problem shape (hardcoded; the harness always runs this config).
B, S, D = 2, 2048, 2048
H, HKV, DH = 32, 8, 64
WIN = 1024
SCALE = 1.0 / float(np.sqrt(DH))

N_CORES = 8
TP = 4                      # head-parallel ways
HLOC = H // TP              # 8 q heads per core
GLOC = HKV // TP            # 2 kv heads per core
EQ = HLOC * DH              # 512 q channels per core
EKV = GLOC * DH             # 128 kv channels per core
NEG_BIG = -1.0e5            # additive mask value (pre-exp, pre-scale)


def _strip_taus(a, nstrip_t, wt):
    """j-tiles contributing to query strip a (4 i-tiles), with their
    valid column range inside the strip.  Returns list of
    (tau, c_lo, c_hi, is_diag, is_edge); a full-coverage tau is first."""
    out = []
    for tau in range(max(0, 4 * a - wt), 4 * a + 4):
        t_lo = max(4 * a, tau)
        t_hi = min(4 * a + 3, tau + wt)
        if t_lo > t_hi or tau >= nstrip_t:
            continue
        c_lo = 128 * t_lo - 512 * a
        c_hi = 128 * (t_hi + 1) - 512 * a
        is_diag = 4 * a <= tau <= 4 * a + 3          # causal block at c_lo
        is_edge = (t_hi == tau + wt)                 # window-edge block at c_hi-128
        out.append((tau, c_lo, c_hi, is_diag, is_edge))
    full = [x for x in out if x[2] - x[1] == 512]
    assert full, f"strip {a} has no full-coverage tau"
    first = full[0]
    return [first] + [x for x in out if x is not first]


def build_program(s=S, d=D, win=WIN, debug=False):
    """Emit the single-core SPMD program.  Returns nc."""
    nt = s // 128           # i/j tiles
    sc_n = s // 512         # 512-wide s chunks
    dc_n = d // 128         # contraction chunks for projections
    wt = win // 128
    nstrip = nt // 4

    nc = bacc.Bacc("TRN2", target_bir_lowering=False, debug=False,
                   num_devices=N_CORES)



    def din(name, shape, dt):
        return nc.dram_tensor(name, shape, dt, kind="ExternalInput").ap()

    # All big tensors are pre-arranged on the host so every DMA reads
    # long contiguous runs per partition.
    xp = din("xp", [sc_n, 128, (d // 128) * 512], F16)
    wqp = din("wqp", [128, (d // 128) * EQ], F16)
    wkp = din("wkp", [128, (d // 128) * EKV], F16)
    wvp = din("wvp", [128, (d // 128) * EKV], F16)
    wop = din("wop", [128, (EQ // 128) * d], F16)
    qaugp = din("qaugp", [4, 64, 2, s], F16)  # [hp, row, u, i]; rows 2+ zero
    kaug = din("kaug", [64, s], F16)          # rows 2+ zero
    mdiag = din("mdiag", [128, 128], F32)     # (jj > ii) * NEG_BIG
    medge = din("medge", [128, 128], F32)     # (jj <= ii) * NEG_BIG
    ident = din("ident", [128, 128], F16)
    out_d = nc.dram_tensor("out", [s // 128, 128, d], F16,
                           kind="ExternalOutput").ap()
    if debug:
        wdump = nc.dram_tensor("wdump", [4, 4, 128, 2, 512], F16,
                               kind="ExternalOutput").ap()
        pvdump = nc.dram_tensor("pvdump", [4, 4, 65, 2, 512], F32,
                                kind="ExternalOutput").ap()
        rcdump = nc.dram_tensor("rcdump", [4, 4, 1, 2, 512], F32,
                                kind="ExternalOutput").ap()
        otdump = nc.dram_tensor("otdump", [4, 128, s], F16,
                                kind="ExternalOutput").ap()

    with tile.TileContext(nc) as tc, ExitStack() as ctx:
        P = ctx.enter_context
        consts = P(tc.tile_pool(name="consts", bufs=1))
        wpool = P(tc.tile_pool(name="wpool", bufs=1))
        xpool = P(tc.tile_pool(name="xpool", bufs=2))
        qapool = P(tc.tile_pool(name="qapool", bufs=1))
        vpool = P(tc.tile_pool(name="vpool", bufs=1))
        otpool = P(tc.tile_pool(name="otpool", bufs=1))
        vtp = P(tc.tile_pool(name="vtp", bufs=2))
        wexp = P(tc.tile_pool(name="wexp", bufs=3))
        nrm = P(tc.tile_pool(name="nrm", bufs=2))
        osbp = P(tc.tile_pool(name="osbp", bufs=3))
        # PSUM: 2 score groups (2 banks each) + pv (2 banks) + fill (2).
        pssc = P(tc.tile_pool(name="pssc", bufs=2, space="PSUM"))
        pspv = P(tc.tile_pool(name="pspv", bufs=1, space="PSUM"))
        psfl = P(tc.tile_pool(name="psfl", bufs=2, space="PSUM"))

        # ---- weights + consts (gpsimd SWDGE queue) ----
        q4w = dc_n // 4
        wq_sb = wpool.tile([128, dc_n, EQ], F16, name="wq_sb")
        nc.gpsimd.dma_start(wq_sb[:], wqp.rearrange("p (c e) -> p c e",
                                                    c=dc_n))
        wk_sb = wpool.tile([128, dc_n, EKV], F16, name="wk_sb")
        nc.gpsimd.dma_start(wk_sb[:], wkp.rearrange("p (c e) -> p c e",
                                                    c=dc_n))
        wv_sb = wpool.tile([128, dc_n, EKV], F16, name="wv_sb")
        nc.gpsimd.dma_start(wv_sb[:], wvp.rearrange("p (c e) -> p c e",
                                                    c=dc_n))
        md_sb = consts.tile([128, 128], F32, name="md_sb")
        nc.gpsimd.dma_start(md_sb[:], mdiag[:])
        me_sb = consts.tile([128, 128], F32, name="me_sb")
        nc.gpsimd.dma_start(me_sb[:], medge[:])
        ident_sb = consts.tile([128, 128], F16, name="ident_sb")
        nc.gpsimd.dma_start(ident_sb[:], ident[:])
        wo_sb = wpool.tile([128, EQ // 128, d], F16, name="wo_sb")
        nc.gpsimd.dma_start(wo_sb[:], wop.rearrange("p (c e) -> p c e",
                                                    c=EQ // 128))

        ones64 = consts.tile([1, 64], F16, name="ones64")
        nc.vector.memset(ones64[:], 1.0)

        # ---- persistent activation tensors ----
        # qa_pair[hp]: [128, 2(u), s] f16; rows 0:64 q values, 64:66 aug.
        qa = []
        for hp in range(4):
            t = qapool.tile([128, 2, s], F16, name=f"qa{hp}")
            nc.gpsimd.dma_start(t[64:128, :, :], qaugp[hp])
            qa.append(t)
        ka = []
        for g in range(GLOC):
            t = qapool.tile([128, s], F16, name=f"ka{g}")
            nc.gpsimd.dma_start(t[64:128, :], kaug[:, :])
            ka.append(t)
        va = []
        for g in range(GLOC):
            t = vpool.tile([128, nt, 128], F16, name=f"va{g}")
            nc.vector.memset(t[:, :, 64:128], 0.0)
            nc.vector.memset(t[:, :, 64:65], 1.0)
            va.append(t)
        oT = []
        for hp in range(4):
            t = otpool.tile([128, s], F16, name=f"oT{hp}")
            oT.append(t)

        # ---------------- filler machinery ----------------
        # Each filler item is (key, generator); generators yield after
        # roughly 1 us of PE work.  drain_through(key) forces everything
        # up to and including that generator to be emitted.
        filler = deque()

        def pump(n_units=1):
            done = 0
            while filler and done < n_units:
                key, gen = filler[0]
                try:
                    next(gen)
                    done += 1
                except StopIteration:
                    filler.popleft()
            return done

        def drain_through(key):
            if not any(k == key for k, _ in filler):
                return
            while filler:
                k0, gen = filler[0]
                for _ in gen:
                    pass
                filler.popleft()
                if k0 == key:
                    break

        def flush_filler():
            while filler:
                _, gen = filler[0]
                for _ in gen:
                    pass
                filler.popleft()

        # ---------------- projection chunk ----------------
        def seed_chunk(sc):
            xt = xpool.tile([128, dc_n, 512], F16, name="xt", tag="xt")
            src = xp[sc].rearrange("p (c s) -> p c s", c=dc_n)
            # split across two DMA queues so quarters land in parallel
            for dq in range(4):
                eng = nc.sync if dq % 2 == 0 else nc.scalar
                eng.dma_start(xt[:, dq * q4w:(dq + 1) * q4w, :],
                              src[:, dq * q4w:(dq + 1) * q4w, :])

            def gen():
                cols = slice(sc * 512, (sc + 1) * 512)
                for et in range(6):
                    ps = psfl.tile([128, 512], F32, name="ps_proj", tag="fl")
                    if et < 4:
                        w_lhs = lambda dc: wq_sb[:, dc, et * 128:(et + 1) * 128]
                    elif et == 4:
                        w_lhs = lambda dc: wk_sb[:, dc, :]
                    else:
                        w_lhs = lambda dc: wv_sb[:, dc, :]
                    for dc4 in range(4):
                        for dc in range(dc4 * 4, dc4 * 4 + 4):
                            nc.tensor.matmul(ps[:], w_lhs(dc), xt[:, dc, :],
                                             start=(dc == 0),
                                             stop=(dc == dc_n - 1))
                        yield
                    if et < 4:
                        nc.vector.tensor_copy(qa[et][0:64, 0, cols],
                                              ps[0:64, :])
                        nc.vector.tensor_copy(qa[et][0:64, 1, cols],
                                              ps[64:128, :])
                    elif et == 4:
                        nc.vector.tensor_copy(ka[0][0:64, cols], ps[0:64, :])
                        nc.vector.tensor_copy(ka[1][0:64, cols], ps[64:128, :])
                    else:
                        vt = vtp.tile([128, 512], F16, name="vt", tag="vt")
                        nc.vector.tensor_copy(vt[:], ps[:])
                        yield
                        for jt in range(4):
                            pst = psfl.tile([128, 128], F16, name="ps_tr",
                                            tag="fl")
                            nc.tensor.transpose(
                                pst[:], vt[:, jt * 128:(jt + 1) * 128],
                                ident_sb[:])
                            jg = sc * 4 + jt
                            nc.vector.tensor_copy(va[0][:, jg, 0:64],
                                                  pst[:, 0:64])
                            nc.vector.tensor_copy(va[1][:, jg, 0:64],
                                                  pst[:, 64:128])
                        yield

            filler.append((("chunk", sc), gen()))

        # ---------------- output projection strip ----------------
        def seed_oproj(a):
            def gen():
                for st in range(4 * a, 4 * a + 4):
                    osb = osbp.tile([128, d], F16, name="osb", tag="osb")
                    for dcb in range(d // 512):
                        ps = psfl.tile([128, 512], F32, name="ps_o", tag="fl")
                        for ec in range(4):
                            nc.tensor.matmul(
                                ps[:], oT[ec][:, st * 128:(st + 1) * 128],
                                wo_sb[:, ec, dcb * 512:(dcb + 1) * 512],
                                start=(ec == 0), stop=(ec == 3))
                        nc.vector.tensor_copy(
                            osb[:, dcb * 512:(dcb + 1) * 512], ps[:])
                        yield
                    nc.sync.dma_start(out_d[st], osb[:])

            filler.append((("oproj", a), gen()))

        # ---------------- attention ----------------
        norm_pending = []

        def flush_norms(keep=0):
            # 1/den = exp(-ln(den)).  Ln and Exp live in different ACT
            # table sets, so batch all Lns then all Exps to pay the table
            # switch twice per flush instead of twice per pair.
            todo = []
            while len(norm_pending) > keep:
                todo.append(norm_pending.pop(0))
            lgs = []
            for (a, hp, pvs) in todo:
                lg = nrm.tile([1, 2, 512], F32, name="lg", tag="lg", bufs=4)
                nc.scalar.activation(lg[:], pvs[64:65, :, :],
                                     mybir.ActivationFunctionType.Ln)
                if debug:
                    nc.sync.dma_start(rcdump[a, hp], lg[:])
                lgs.append(lg)
            for (a, hp, pvs), lg in zip(todo, lgs):
                rc16 = nrm.tile([1, 2, 512], F16, name="rc16", tag="rc16",
                                bufs=4)
                nc.scalar.activation(rc16[:], lg[:],
                                     mybir.ActivationFunctionType.Exp,
                                     scale=-1.0)
                for u in range(2):
                    rbp = psfl.tile([64, 512], F32, name="rbp", tag="fl")
                    nc.tensor.matmul(rbp[:], ones64[:], rc16[:, u, :],
                                     start=True, stop=True)
                    nc.vector.tensor_mul(
                        oT[hp][u * 64:(u + 1) * 64,
                               a * 512:(a + 1) * 512],
                        pvs[0:64, u, :], rbp[:])

        pend = deque()   # PV pipeline; crosses pair boundaries

        def drain_one():
            (a, hp, pv, ptau, first_tau, last, pc_lo, pc_hi, pw, pn) = \
                pend.popleft()
            g = hp // 2
            for u in range(2):
                nc.tensor.matmul(pv[:, u, pc_lo:pc_hi],
                                 va[g][:, ptau, :], pw[:, u, 0:pn],
                                 start=(ptau == first_tau), stop=last)
            if last:
                # evacuate PV to SBUF right away so the next pair's PV
                # matmuls don't wait on the deferred normalization chain.
                pvs = nrm.tile([65, 2, 512], F32, name="pvs", tag="pvs",
                               bufs=5)
                nc.vector.tensor_copy(pvs[:], pv[0:65, :, :])
                if debug:
                    nc.sync.dma_start(pvdump[a, hp], pvs[:])
                norm_pending.append((a, hp, pvs))

        def emit_attn_pair(a, hp):
            g = hp // 2
            taus = _strip_taus(a, nt, wt)
            pv = pspv.tile([128, 2, 512], F32, name="pv", tag="pv")
            last_tau = taus[-1][0]
            for idx, (tau, c_lo, c_hi, is_diag, is_edge) in enumerate(taus):
                n = c_hi - c_lo
                pss = pssc.tile([128, 2, 512], F32, name="pss", tag="sc")
                for u in range(2):
                    nc.tensor.matmul(
                        pss[:, u, 0:n],
                        ka[g][:, tau * 128:(tau + 1) * 128],
                        qa[hp][:, u, 512 * a + c_lo:512 * a + c_hi],
                        start=True, stop=True)
                if is_diag:
                    for u in range(2):
                        nc.vector.tensor_add(pss[:, u, 0:128],
                                             pss[:, u, 0:128], md_sb[:])
                if is_edge:
                    for u in range(2):
                        nc.vector.tensor_add(pss[:, u, n - 128:n],
                                             pss[:, u, n - 128:n], me_sb[:])
                w_t = wexp.tile([128, 2, 512], F16, name="w_t", tag="w")
                nc.scalar.activation(
                    w_t[:, :, 0:n], pss[:, :, 0:n],
                    mybir.ActivationFunctionType.Exp, scale=SCALE)
                if debug and a == 0:
                    nc.sync.dma_start(wdump[hp, tau, :, :, 0:n],
                                      w_t[:, :, 0:n])
                pend.append((a, hp, pv, tau, taus[0][0], tau == last_tau,
                             c_lo, c_hi, w_t, n))
                if len(pend) > 2:
                    drain_one()
                pump(1)

        # ---------------- schedule ----------------
        # Warm the PE HAM clock gate during the initial DMA wait: ~80
        # dummy matmuls on a zeroed tile keep the PE busy so the real
        # projection stream starts at 2.4 GHz instead of 1.2 GHz.
        warm = consts.tile([128, 128], F16, name="warm")
        nc.vector.memset(warm[:], 0.0)
        for wgrp in range(10):
            wps = psfl.tile([128, 128], F32, name="wps", tag="fl")
            for wi in range(8):
                nc.tensor.matmul(wps[:], warm[:], warm[:],
                                 start=(wi == 0), stop=(wi == 7))
        seed_chunk(0)
        drain_through(("chunk", 0))
        for a in range(nstrip):
            if a + 1 < sc_n:
                seed_chunk(a + 1)
            drain_through(("chunk", a))
            for hp in range(4):
                emit_attn_pair(a, hp)
            # drain the PV pipeline for this strip so its norms exist
            # before the output projection is seeded.
            while pend:
                drain_one()
            flush_norms()
            seed_oproj(a)
        flush_filler()
        if debug:
            for hp in range(4):
                nc.sync.dma_start(otdump[hp], oT[hp][:])

    nc.compile()
    return nc


# ---------------- host-side sharding ----------------

def _prep_core_inputs(c, x, Wq, Wk, Wv, Wo, slopes, s=S, d=D):
    """Build the per-core input map (all numpy, fp16 where declared)."""
    b = c // TP
    hs = c % TP
    f16 = np.float16
    qrows = slice(hs * EQ, (hs + 1) * EQ)
    krows = slice(hs * EKV, (hs + 1) * EKV)
    dc_n = d // 128
    m = {}
    # xp[sc, p, c*512+ss] = x[b, sc*512+ss, c*128+p]
    xT = x[b].T.astype(f16)                       # [d, s]
    xp = xT.reshape(dc_n, 128, s // 512, 512).transpose(2, 1, 0, 3)
    m["xp"] = np.ascontiguousarray(xp).reshape(s // 512, 128, dc_n * 512)
    # w*[p, c*E+e] = W[e_global, c*128+p].T
    wq = Wq[qrows, :].T.astype(f16).reshape(dc_n, 128, EQ)
    m["wqp"] = np.ascontiguousarray(wq.transpose(1, 0, 2)).reshape(128, -1)
    wk = Wk[krows, :].T.astype(f16).reshape(dc_n, 128, EKV)
    m["wkp"] = np.ascontiguousarray(wk.transpose(1, 0, 2)).reshape(128, -1)
    wv = Wv[krows, :].T.astype(f16).reshape(dc_n, 128, EKV)
    m["wvp"] = np.ascontiguousarray(wv.transpose(1, 0, 2)).reshape(128, -1)
    wo = Wo[:, qrows].T.astype(f16).reshape(EQ // 128, 128, d)
    m["wop"] = np.ascontiguousarray(wo.transpose(1, 0, 2)).reshape(128, -1)
    i_idx = np.arange(s, dtype=np.float32)
    qaugp = np.zeros((4, 64, 2, s), np.float32)
    for hp in range(4):
        for u in range(2):
            sl = float(slopes[hs * HLOC + 2 * hp + u])
            qaugp[hp, 0, u, :] = sl / SCALE
            qaugp[hp, 1, u, :] = -sl / SCALE * i_idx
    m["qaugp"] = qaugp.astype(f16)
    kaug = np.zeros((64, s), np.float32)
    kaug[0, :] = i_idx
    kaug[1, :] = 1.0
    m["kaug"] = kaug.astype(f16)
    m["ident"] = np.eye(128, dtype=f16)
    p = np.arange(128)[:, None]
    f = np.arange(128)[None, :]
    m["mdiag"] = ((p > f) * NEG_BIG).astype(np.float32)
    m["medge"] = ((p <= f) * NEG_BIG).astype(np.float32)
    return m


_PROG_CACHE = {}


def _get_program():
    key = (S, D, WIN)
    if key not in _PROG_CACHE:
        _PROG_CACHE[key] = build_program()
    return _PROG_CACHE[key]


def kernel(hidden_states, Wq, bq, Wk, bk, Wv, bv, Wo, bo, alibi_slopes,
           _want_profile=False):
    x = np.asarray(hidden_states, np.float32)
    Wq = np.asarray(Wq, np.float32)
    Wk = np.asarray(Wk, np.float32)
    Wv = np.asarray(Wv, np.float32)
    Wo = np.asarray(Wo, np.float32)
    bq = np.asarray(bq, np.float32)
    bv = np.asarray(bv, np.float32)
    bo = np.asarray(bo, np.float32)
    slopes = np.asarray(alibi_slopes, np.float32)

    # bq shifts scores by (Wk^T bq). x_j, which does not cancel in the
    # softmax; the device path assumes it is zero (true for this problem).
    assert np.abs(bq).max() < 1e-6, "nonzero bq not supported"
    # bk adds a per-query constant to every in-window logit -> cancels in
    # softmax.  bv adds a constant per v channel; probs sum to 1 so it
    # shifts o by bv -> fold (bv_expanded @ Wo.T + bo) into the output.
    group = H // HKV
    bv_exp = np.repeat(np.asarray(bv, np.float32).reshape(HKV, DH),
                       group, axis=0).reshape(-1)
    out_const = bv_exp @ Wo.T + bo

    nc = _get_program()
    in_maps = [
        _prep_core_inputs(c, x, Wq, Wk, Wv, Wo, slopes)
        for c in range(N_CORES)
    ]
    res = run_bass_kernel_spmd(nc, in_maps, list(range(N_CORES)),
                               trace=_want_profile)
    out = np.zeros((B, S, D), np.float32)
    for c in range(N_CORES):
        out[c // TP] += res.results[c]["out"].astype(np.float32).reshape(S, D)
    out += out_const[None, None, :]
    if _want_profile:
        return out, res
    return out
